# revision 9
# baseline (speedup 1.0000x reference)
"""Trainium2 Bass kernel for CachedMultiHeadedAttention (tensor-parallel over heads).

Sharding: 8 cores x 4 heads. Each core computes Q projection + attention for
its 4 heads, then a partial output projection against its 512 rows of Wo.
Host sums the 8 partial outputs (the "all-reduce" done at unshard time) and
adds bo.

Device-side layouts are chosen so NO on-chip transposes are needed:
  - x is passed pre-transposed (xT [D, S]) so contraction dims land on
    SBUF partitions for every matmul.
  - k_cache is passed pre-transposed per head (kT [DK, pos]).
  - The reference's softmax quirk (softmax over the QUERY axis) maps to
    scores^T tiles [l_part, s_free]: one fused ACT pass does exp + row-sum.
    The 1/sum normalization is folded into V rows (8x less data than the
    weight matrix).
All matmuls run as float32r (full PE rate; plain fp32 matmul is 1/4 rate).
"""

import math
import os
from contextlib import ExitStack

import numpy as np

import concourse.bass as bass
import concourse.mybir as mybir
import concourse.tile as tile
from concourse import bacc
from concourse.bass_utils import run_bass_kernel_spmd

F32 = mybir.dt.float32
F32R = mybir.dt.float32r
AF = mybir.ActivationFunctionType

H, D, DK, S = 32, 4096, 128, 1024
NCORES = 8
HP = H // NCORES          # heads per core
DC = D // 128             # contraction chunks for d_model


def build(pos: int):
    L = pos + 1
    LC = (L + 127) // 128          # number of 128-wide l tiles
    INV = 1.0 / math.sqrt(DK)

    nc = bacc.Bacc("TRN2", target_bir_lowering=False, debug=False,
                   num_devices=NCORES)

    xT_d = nc.dram_tensor("xT", [D, S], F32R, kind="ExternalInput").ap()
    wq_d = nc.dram_tensor("wq", [HP, D, DK], F32R, kind="ExternalInput").ap()
    wkv_d = nc.dram_tensor("wkv", [D, 2 * HP * DK], F32R, kind="ExternalInput").ap()
    bq_d = nc.dram_tensor("bq", [HP, DK, 1], F32, kind="ExternalInput").ap()
    bkv_d = nc.dram_tensor("bkv", [1, 2 * HP * DK], F32, kind="ExternalInput").ap()
    kT_d = nc.dram_tensor("kT", [HP, DK, pos], F32R, kind="ExternalInput").ap()
    v_d = nc.dram_tensor("v", [HP, pos, DK], F32R, kind="ExternalInput").ap()
    wo_d = nc.dram_tensor("wo", [HP * DK, D], F32R, kind="ExternalInput").ap()
    out_d = nc.dram_tensor("out", [S, D], F32, kind="ExternalOutput").ap()

    with tile.TileContext(nc) as tc:
        # ---- pools for the attention phases ----
        # Pools are released LIFO; ctxT survives into the output projection,
        # so it sits at the bottom of the SBUF pool stack.
        ctxT_pool = tc.alloc_tile_pool(name="ctxT", bufs=1)
        xT_pool = tc.alloc_tile_pool(name="xT", bufs=1)
        qT_pool = tc.alloc_tile_pool(name="qT", bufs=2)
        small = tc.alloc_tile_pool(name="smallp", bufs=1)
        wq_pool = tc.alloc_tile_pool(name="wqp", bufs=8)
        wkv_pool = tc.alloc_tile_pool(name="wkvp", bufs=4)
        kt_pool = tc.alloc_tile_pool(name="ktp", bufs=6)
        v_pool = tc.alloc_tile_pool(name="vp", bufs=6)
        wt_pool = tc.alloc_tile_pool(name="wtp", bufs=5)
        vs_pool = tc.alloc_tile_pool(name="vsp", bufs=4)
        ss_pool = tc.alloc_tile_pool(name="ssp", bufs=8)

        psq = tc.alloc_tile_pool(name="psq", bufs=1, space="PSUM")
        pss = tc.alloc_tile_pool(name="pss", bufs=3, space="PSUM")
        psc = tc.alloc_tile_pool(name="psc", bufs=1, space="PSUM")

        # resident xT tiles: chunk c holds xT[c*128:(c+1)*128, :]
        xts = []
        for c in range(DC):
            xt = xT_pool.tile([128, S], F32R, name=f"xt{c}", tag=f"xt{c}")
            nc.sync.dma_start(xt[:], xT_d[c * 128:(c + 1) * 128, :])
            xts.append(xt)

        ctxTs = [ctxT_pool.tile([128, S], F32R, name=f"cT{h}", tag=f"cT{h}")
                 for h in range(HP)]

        # biased k_new|v_new rows for all heads ([1, 2*HP*DK]), built once
        # during head 0's Q phase
        kvrow = small.tile([1, 2 * HP * DK], F32R, name="kvrow", tag="kvrow")
        bkv_t = small.tile([1, 2 * HP * DK], F32, name="bkvt", tag="bkvt")
        nc.sync.dma_start(bkv_t[:], bkv_d[:])

        for h in range(HP):
            # ---------- Q projection for head h (+ k_new col, + v_new row) ----------
            psq_t = psq.tile([128, S], F32, name=f"psq{h}", tag="psq")
            if h == 0:
                # kv_new rows borrow pss slots (pss is idle during head 0's Q)
                kn_t = pss.tile([1, HP * DK], F32, name="knr", tag="pss")
                vn_t = pss.tile([1, HP * DK], F32, name="vnr", tag="pss")
            for c in range(DC):
                wqt = wq_pool.tile([128, DK], F32R, name=f"wq{h}_{c}", tag="wq")
                nc.sync.dma_start(wqt[:], wq_d[h, c * 128:(c + 1) * 128, :])
                st, sp = (c == 0), (c == DC - 1)
                nc.tensor.matmul(psq_t[:, 0:512], (wqt[:]), (xts[c][:, 0:512]),
                                 start=st, stop=sp)
                nc.tensor.matmul(psq_t[:, 512:1024], (wqt[:]), (xts[c][:, 512:1024]),
                                 start=st, stop=sp)
                if h == 0:
                    wkvt = wkv_pool.tile([128, 2 * HP * DK], F32R,
                                         name=f"wkv{c}", tag="wkv")
                    nc.sync.dma_start(wkvt[:], wkv_d[c * 128:(c + 1) * 128, :])
                    nc.tensor.matmul(kn_t[0:1, :], (xts[c][:, S - 1:S]),
                                     (wkvt[:, 0:HP * DK]), start=st, stop=sp)
                    nc.tensor.matmul(vn_t[0:1, :], (xts[c][:, S - 1:S]),
                                     (wkvt[:, HP * DK:]), start=st, stop=sp)
            # qT = psq + bq  (per-partition bias, fused into the PSUM->SBUF copy)
            bq_t = ss_pool.tile([128, 1], F32, name=f"bq{h}", tag="bq", bufs=2)
            nc.sync.dma_start(bq_t[:], bq_d[h])
            qT_t = qT_pool.tile([128, S], F32R, name=f"qT{h}", tag="qT")
            nc.scalar.activation(qT_t[:], psq_t[:], AF.Identity, bias=bq_t[:])
            if h == 0:
                nc.vector.tensor_add(kvrow[0:1, 0:HP * DK], kn_t[:],
                                     bkv_t[0:1, 0:HP * DK])
                nc.vector.tensor_add(kvrow[0:1, HP * DK:], vn_t[:],
                                     bkv_t[0:1, HP * DK:])

            # ---------- attention for head h over l tiles ----------
            psc_t = psc.tile([128, S], F32, name=f"psc{h}", tag="psc")
            for lt in range(LC):
                l0 = lt * 128
                lsz = min(128, L - l0)            # valid l rows in this tile
                csz = max(0, min(128, pos - l0))  # of which from the cache
                ktile = kt_pool.tile([128, 128], F32R, name=f"kt{h}_{lt}", tag="kt")
                if csz > 0:
                    nc.sync.dma_start(ktile[:, 0:csz], kT_d[h, :, l0:l0 + csz])
                if lsz > csz:  # the new entry at l == pos
                    nc.sync.dma_start(ktile[:, csz:csz + 1],
                                      kvrow[0:1, h * DK:(h + 1) * DK])
                if lsz < 128:
                    nc.vector.memset(ktile[:, lsz:128], 0.0)
                vtile = v_pool.tile([128, DK], F32R, name=f"v{h}_{lt}", tag="v")
                if csz > 0:
                    nc.sync.dma_start(vtile[0:csz, :], v_d[h, l0:l0 + csz, :])
                if lsz > csz:
                    nc.sync.dma_start(vtile[csz:csz + 1, :],
                                      kvrow[0:1, HP * DK + h * DK:
                                            HP * DK + (h + 1) * DK])
                if lsz < 128:
                    nc.vector.memset(vtile[lsz:128, :], 0.0)

                ps0 = pss.tile([128, 512], F32, name=f"ps0_{h}_{lt}", tag="pss")
                ps1 = pss.tile([128, 512], F32, name=f"ps1_{h}_{lt}", tag="pss")
                nc.tensor.matmul(ps0[:], (ktile[:]), (qT_t[:, 0:512]))
                nc.tensor.matmul(ps1[:], (ktile[:]), (qT_t[:, 512:1024]))

                wt0 = wt_pool.tile([128, 512], F32R, name=f"wt0_{h}_{lt}", tag="wt")
                wt1 = wt_pool.tile([128, 512], F32R, name=f"wt1_{h}_{lt}", tag="wt")
                s0 = ss_pool.tile([128, 1], F32, name=f"s0_{h}_{lt}", tag="s0")
                s1 = ss_pool.tile([128, 1], F32, name=f"s1_{h}_{lt}", tag="s1")
                nc.scalar.activation(wt0[:], ps0[:], AF.Exp, scale=INV, accum_out=s0[:])
                nc.scalar.activation(wt1[:], ps1[:], AF.Exp, scale=INV, accum_out=s1[:])
                ssum = ss_pool.tile([128, 1], F32, name=f"ss_{h}_{lt}", tag="ssum")
                nc.vector.tensor_add(ssum[:], s0[:], s1[:])
                rec = ss_pool.tile([128, 1], F32, name=f"rc_{h}_{lt}", tag="rec")
                nc.vector.reciprocal(rec[:], ssum[:])
                vst = vs_pool.tile([128, DK], F32R, name=f"vs{h}_{lt}", tag="vs")
                nc.vector.tensor_scalar_mul(vst[:], vtile[:], rec[:])

                st, sp = (lt == 0), (lt == LC - 1)
                nc.tensor.matmul(psc_t[:, 0:512], (vst[:]), (wt0[:]),
                                 start=st, stop=sp)
                nc.tensor.matmul(psc_t[:, 512:1024], (vst[:]), (wt1[:]),
                                 start=st, stop=sp)
            nc.scalar.activation(ctxTs[h][:], psc_t[:], AF.Copy, bias=0.0)

        # release attention-phase pools before the output projection (LIFO)
        for p in (psc, pss, psq,
                  ss_pool, vs_pool, wt_pool, v_pool, kt_pool,
                  wkv_pool, wq_pool, small, qT_pool, xT_pool):
            p.release()

        # ---------- output projection: out[s, m] partial ----------
        wo_pool = tc.alloc_tile_pool(name="wop", bufs=2)
        pso = tc.alloc_tile_pool(name="pso", bufs=4, space="PSUM")
        ob_pool = tc.alloc_tile_pool(name="obp", bufs=4)
        MG = D // 512                      # 8 m-groups of 512 columns
        for mg in range(MG):
            wts = []
            for c in range(HP):
                wot = wo_pool.tile([128, 512], F32R, name=f"wo{mg}_{c}", tag=f"wo{c}")
                nc.sync.dma_start(wot[:], wo_d[c * 128:(c + 1) * 128,
                                               mg * 512:(mg + 1) * 512])
                wts.append(wot)
            for s_t in range(S // 128):
                pso_t = pso.tile([128, 512], F32, name=f"po{mg}_{s_t}", tag="pso")
                for c in range(HP):
                    nc.tensor.matmul(pso_t[:],
                                     (ctxTs[c][:, s_t * 128:(s_t + 1) * 128]),
                                     (wts[c][:]),
                                     start=(c == 0), stop=(c == HP - 1))
                ot = ob_pool.tile([128, 512], F32, name=f"ot{mg}_{s_t}", tag="ot")
                nc.scalar.activation(ot[:], pso_t[:], AF.Copy, bias=0.0)
                nc.sync.dma_start(out_d[s_t * 128:(s_t + 1) * 128,
                                        mg * 512:(mg + 1) * 512], ot[:])
        for p in (pso, ob_pool, wo_pool, ctxT_pool):
            p.release()

    nc.compile()
    return nc


_CACHE = {}
LAST_EXEC_NS = None


def kernel(x, k_cache, v_cache, Wq, bq, Wk, bk, Wv, bv, Wo, bo, pos):
    global LAST_EXEC_NS
    pos = int(pos)

    def f32(a):
        return np.ascontiguousarray(np.asarray(a), dtype=np.float32)

    x = f32(x)
    k_cache, v_cache = f32(k_cache), f32(v_cache)
    Wq, Wk, Wv, Wo = f32(Wq), f32(Wk), f32(Wv), f32(Wo)
    bq, bk, bv, bo = f32(bq), f32(bk), f32(bv), f32(bo)

    xT = np.ascontiguousarray(x[0].T)              # [D, S]
    in_maps = []
    for i in range(NCORES):
        hs = slice(i * HP, (i + 1) * HP)
        in_maps.append({
            "xT": xT,
            "wq": np.ascontiguousarray(Wq[hs]),
            "wkv": np.ascontiguousarray(np.concatenate([
                Wk[hs].transpose(1, 0, 2).reshape(D, HP * DK),
                Wv[hs].transpose(1, 0, 2).reshape(D, HP * DK)], axis=1)),
            "bq": np.ascontiguousarray(bq[hs].reshape(HP, DK, 1)),
            "bkv": np.ascontiguousarray(np.concatenate(
                [bk[hs].reshape(-1), bv[hs].reshape(-1)])[None, :]),
            "kT": np.ascontiguousarray(k_cache[hs, :pos, :].transpose(0, 2, 1)),
            "v": np.ascontiguousarray(v_cache[hs, :pos, :]),
            "wo": np.ascontiguousarray(Wo[i * HP * DK:(i + 1) * HP * DK]),
        })

    if pos not in _CACHE:
        _CACHE[pos] = build(pos)
    nc = _CACHE[pos]

    res = run_bass_kernel_spmd(nc, in_maps, core_ids=list(range(NCORES)))
    LAST_EXEC_NS = res.exec_time_ns

    acc = np.zeros((S, D), np.float64)
    for r in res.results:
        acc += r["out"]
    out = (acc + bo.astype(np.float64)).astype(np.float32)
    return out[None]


# revision 13
# speedup vs baseline: 1.2402x; 1.2402x over previous
"""Trainium2 Bass kernel for CachedMultiHeadedAttention (tensor-parallel over heads).

Sharding: 8 cores x 4 heads. Each core computes Q projection + attention for
its 4 heads, then a partial output projection against its 512 rows of Wo.
Host sums the 8 partial outputs (the "all-reduce" done at unshard time) and
adds bo.

Device-side layouts are chosen so NO on-chip transposes are needed:
  - x is passed pre-transposed (xT [D, S]) so contraction dims land on
    SBUF partitions for every matmul.
  - k_cache is passed pre-transposed per head (kT [DK, pos]).
  - The reference's softmax quirk (softmax over the QUERY axis) maps to
    scores^T tiles [l_part, s_free]: one fused ACT pass does exp + row-sum.
    The 1/sum normalization is folded into V rows (8x less data than the
    weight matrix).
Matmuls run as float32r (full PE rate; plain fp32 matmul is 1/4 rate).
The k_new/v_new projections (rank-1 work) run in bf16 — they only affect one
of the 4096 cache rows, so their rounding is negligible in the output.

DMAs are consolidated into few large transfers: each dma_start costs ~625ns
on the single HWDGE queue, which was the dominant bottleneck of the naive
version (557 DMAs = 348us of serialized HWDGE time).
"""

import math

import numpy as np
import ml_dtypes

import concourse.bass as bass
import concourse.mybir as mybir
import concourse.tile as tile
from concourse import bacc
from concourse.bass_utils import run_bass_kernel_spmd

F32 = mybir.dt.float32
F32R = mybir.dt.float32r
BF16 = mybir.dt.bfloat16
AF = mybir.ActivationFunctionType

H, D, DK, S = 32, 4096, 128, 1024
NCORES = 8
HP = H // NCORES          # heads per core
DC = D // 128             # contraction chunks for d_model


def build(pos: int):
    L = pos + 1
    LC = (L + 127) // 128          # number of 128-wide l tiles
    LG = (LC + 7) // 8             # l-tile groups of 8 (1024 l per group)
    INV = 1.0 / math.sqrt(DK)

    nc = bacc.Bacc("TRN2", target_bir_lowering=False, debug=False,
                   num_devices=NCORES)

    xT_d = nc.dram_tensor("xT", [D, S], F32R, kind="ExternalInput").ap()
    wq_d = nc.dram_tensor("wq", [HP, D, DK], F32R, kind="ExternalInput").ap()
    wkv_d = nc.dram_tensor("wkv", [D, 2 * HP * DK], BF16, kind="ExternalInput").ap()
    xl_d = nc.dram_tensor("xl", [128, DC], BF16, kind="ExternalInput").ap()
    bq_d = nc.dram_tensor("bq", [HP, DK, 1], F32, kind="ExternalInput").ap()
    bkv_d = nc.dram_tensor("bkv", [1, 2 * HP * DK], F32, kind="ExternalInput").ap()
    kT_d = nc.dram_tensor("kT", [HP, DK, pos], F32R, kind="ExternalInput").ap()
    v_d = nc.dram_tensor("v", [HP, pos, DK], F32R, kind="ExternalInput").ap()
    wo_d = nc.dram_tensor("wo", [HP * DK, D], F32R, kind="ExternalInput").ap()
    out_d = nc.dram_tensor("out", [S, D], F32, kind="ExternalOutput").ap()

    with tile.TileContext(nc) as tc:
        # Pools are released LIFO; ctxT survives into the output projection,
        # so it sits at the bottom of the SBUF pool stack.
        ctxT_pool = tc.alloc_tile_pool(name="ctxT", bufs=1)
        xT_pool = tc.alloc_tile_pool(name="xT", bufs=1)
        qT_pool = tc.alloc_tile_pool(name="qT", bufs=2)
        small = tc.alloc_tile_pool(name="smallp", bufs=1)
        wq_pool = tc.alloc_tile_pool(name="wqp", bufs=3)
        wkv_pool = tc.alloc_tile_pool(name="wkvp", bufs=2)
        kt_pool = tc.alloc_tile_pool(name="ktp", bufs=2)
        v_pool = tc.alloc_tile_pool(name="vp", bufs=2)
        wt_pool = tc.alloc_tile_pool(name="wtp", bufs=4)
        vs_pool = tc.alloc_tile_pool(name="vsp", bufs=4)
        ss_pool = tc.alloc_tile_pool(name="ssp", bufs=8)

        psq = tc.alloc_tile_pool(name="psq", bufs=1, space="PSUM")
        pss = tc.alloc_tile_pool(name="pss", bufs=2, space="PSUM")
        psc = tc.alloc_tile_pool(name="psc", bufs=1, space="PSUM")

        # resident xT tiles: 8 big tiles of 4 chunks each (4KB descriptors)
        xbig = []
        for gx in range(DC // 4):
            xt = xT_pool.tile([128, 4 * S], F32R, name=f"xt{gx}", tag=f"xt{gx}")
            nc.sync.dma_start(
                xt[:], xT_d[gx * 512:(gx + 1) * 512, :].rearrange(
                    "(i p) s -> p i s", p=128))
            xbig.append(xt)

        def xsl(c, lo, sz):
            return xbig[c // 4][:, (c % 4) * S + lo:(c % 4) * S + lo + sz]

        ctxTs = [ctxT_pool.tile([128, S], F32R, name=f"cT{h}", tag=f"cT{h}")
                 for h in range(HP)]

        # biased k_new|v_new rows for all heads ([1, 2*HP*DK])
        kvrow = small.tile([1, 2 * HP * DK], F32R, name="kvrow", tag="kvrow")
        bkv_t = small.tile([1, 2 * HP * DK], F32, name="bkvt", tag="bkvt")
        nc.sync.dma_start(bkv_t[:], bkv_d[:])
        # x_last chunks in bf16: column c = x[-1, c*128:(c+1)*128]
        xl_t = small.tile([128, DC], BF16, name="xlt", tag="xlt")
        nc.sync.dma_start(xl_t[:], xl_d[:])

        # k_new/v_new psum accumulators (rows, all heads)
        kn_t = pss.tile([1, HP * DK], F32, name="knr", tag="kn", bufs=1)
        vn_t = pss.tile([1, HP * DK], F32, name="vnr", tag="vn", bufs=1)

        npos_g = pos // 1024            # l-group containing the new entry
        npos_j = (pos % 1024) // 128    # l-tile within that group
        for h in range(HP):
            # ---------- Q projection for head h ----------
            psq_t = psq.tile([128, S], F32, name=f"psq{h}", tag="psq")
            wqts = []
            for gw in range(DC // 4):   # 8 weight groups of 4 chunks
                wqt = wq_pool.tile([128, 4 * DK], F32R,
                                   name=f"wq{h}_{gw}", tag="wq")
                nc.sync.dma_start(
                    wqt[:], wq_d[h, gw * 512:(gw + 1) * 512, :].rearrange(
                        "(i p) k -> p i k", p=128))
                wqts.append(wqt)
            for c in range(DC):
                lhs = wqts[c // 4][:, (c % 4) * DK:(c % 4 + 1) * DK]
                st, sp = (c == 0), (c == DC - 1)
                nc.tensor.matmul(psq_t[:, 0:512], lhs, xsl(c, 0, 512),
                                 start=st, stop=sp)
                nc.tensor.matmul(psq_t[:, 512:1024], lhs, xsl(c, 512, 512),
                                 start=st, stop=sp)
            # qT = psq + bq (per-partition bias) on DVE
            bq_t = ss_pool.tile([128, 1], F32, name=f"bq{h}", tag="bq", bufs=2)
            nc.sync.dma_start(bq_t[:], bq_d[h])
            qT_t = qT_pool.tile([128, S], F32R, name=f"qT{h}", tag="qT")
            nc.vector.tensor_scalar_add(qT_t[:], psq_t[:], bq_t[:])

            if h == 0:
                # rank-1 k_new/v_new projections (bf16) — emitted before the
                # S loop so the kvrow write precedes all kvrow readers in
                # trace order (Tile tracks deps in emission order).
                for c in range(DC):
                    wkvt = wkv_pool.tile([128, 2 * HP * DK], BF16,
                                         name=f"wkv{c}", tag="wkv")
                    nc.sync.dma_start(wkvt[:], wkv_d[c * 128:(c + 1) * 128, :])
                    st, sp = (c == 0), (c == DC - 1)
                    nc.tensor.matmul(kn_t[0:1, :], xl_t[:, c:c + 1],
                                     wkvt[:, 0:HP * DK], start=st, stop=sp)
                    nc.tensor.matmul(vn_t[0:1, :], xl_t[:, c:c + 1],
                                     wkvt[:, HP * DK:], start=st, stop=sp)
                nc.vector.tensor_add(kvrow[0:1, 0:HP * DK], kn_t[:],
                                     bkv_t[0:1, 0:HP * DK])
                nc.vector.tensor_add(kvrow[0:1, HP * DK:], vn_t[:],
                                     bkv_t[0:1, HP * DK:])

            # ---------- attention for head h over l tiles ----------
            psc_t = psc.tile([128, S], F32, name=f"psc{h}", tag="psc")
            kt8 = v8 = None
            for lt in range(LC):
                g, j = lt // 8, lt % 8
                if j == 0:
                    # load l-group g: keys (16KB bursts) and values (512B)
                    g0 = g * 1024
                    gl = min(1024, L - g0)            # valid l in group
                    gc = max(0, min(1024, pos - g0))  # from cache
                    kt8 = kt_pool.tile([128, 1024], F32R,
                                       name=f"kt{h}_{g}", tag="kt")
                    if gc > 0:
                        nc.sync.dma_start(kt8[:, 0:gc], kT_d[h, :, g0:g0 + gc])
                    if gl > gc:   # new entry column
                        nc.sync.dma_start(kt8[:, gc:gc + 1],
                                          kvrow[0:1, h * DK:(h + 1) * DK])
                    if gl < 1024:
                        nc.vector.memset(kt8[:, gl:1024], 0.0)
                    v8 = v_pool.tile([128, 1024], F32R,
                                     name=f"v{h}_{g}", tag="v")
                    fc = gc // 128                    # full cache chunks
                    if fc > 0:
                        nc.sync.dma_start(
                            v8[:, 0:fc * 128],
                            v_d[h, g0:g0 + fc * 128, :].rearrange(
                                "(i p) k -> p i k", p=128))
                    rem = gc - fc * 128               # partial cache chunk
                    if rem > 0:
                        nc.sync.dma_start(
                            v8[0:rem, fc * 128:(fc + 1) * 128],
                            v_d[h, g0 + fc * 128:g0 + gc, :])
                    if gl > gc:   # new entry row
                        nc.sync.dma_start(
                            v8[gc % 128:gc % 128 + 1,
                               (gc // 128) * 128:(gc // 128 + 1) * 128],
                            kvrow[0:1, HP * DK + h * DK:HP * DK + (h + 1) * DK])
                    if gl < 1024:
                        for cc in range(gl // 128, 8):
                            lo = max(gl - cc * 128, 0)
                            if lo < 128:
                                nc.vector.memset(v8[lo:128, cc * 128:(cc + 1) * 128], 0.0)

                ps0 = pss.tile([128, 512], F32, name=f"ps0_{h}_{lt}", tag="pss")
                ps1 = pss.tile([128, 512], F32, name=f"ps1_{h}_{lt}", tag="pss")
                ksl = kt8[:, j * 128:(j + 1) * 128]
                nc.tensor.matmul(ps0[:], ksl, qT_t[:, 0:512])
                nc.tensor.matmul(ps1[:], ksl, qT_t[:, 512:1024])

                wt0 = wt_pool.tile([128, 512], F32R, name=f"wt0_{h}_{lt}", tag="wt")
                wt1 = wt_pool.tile([128, 512], F32R, name=f"wt1_{h}_{lt}", tag="wt")
                s0 = ss_pool.tile([128, 1], F32, name=f"s0_{h}_{lt}", tag="s0")
                s1 = ss_pool.tile([128, 1], F32, name=f"s1_{h}_{lt}", tag="s1")
                nc.scalar.activation(wt0[:], ps0[:], AF.Exp, scale=INV, accum_out=s0[:])
                nc.scalar.activation(wt1[:], ps1[:], AF.Exp, scale=INV, accum_out=s1[:])
                ssum = ss_pool.tile([128, 1], F32, name=f"ss_{h}_{lt}", tag="ssum")
                nc.vector.tensor_add(ssum[:], s0[:], s1[:])
                rec = ss_pool.tile([128, 1], F32, name=f"rc_{h}_{lt}", tag="rec")
                nc.vector.reciprocal(rec[:], ssum[:])
                vst = vs_pool.tile([128, DK], F32R, name=f"vs{h}_{lt}", tag="vs")
                nc.vector.tensor_scalar_mul(vst[:], v8[:, j * 128:(j + 1) * 128], rec[:])

                st, sp = (lt == 0), (lt == LC - 1)
                nc.tensor.matmul(psc_t[:, 0:512], vst[:], wt0[:],
                                 start=st, stop=sp)
                nc.tensor.matmul(psc_t[:, 512:1024], vst[:], wt1[:],
                                 start=st, stop=sp)
            nc.vector.tensor_copy(ctxTs[h][:], psc_t[:])

        # release attention-phase pools before the output projection (LIFO)
        for p in (psc, pss, psq,
                  ss_pool, vs_pool, wt_pool, v_pool, kt_pool,
                  wkv_pool, wq_pool, small, qT_pool, xT_pool):
            p.release()

        # ---------- output projection: out[s, m] partial ----------
        # Wo fully resident in the space freed by xT; one 16KB-burst output
        # DMA per s-tile.
        wo_pool = tc.alloc_tile_pool(name="wop", bufs=1)
        ob_pool = tc.alloc_tile_pool(name="obp", bufs=2)
        pso = tc.alloc_tile_pool(name="pso", bufs=4, space="PSUM")
        wos = []
        for c in range(HP):
            wot = wo_pool.tile([128, D], F32R, name=f"wo{c}", tag=f"wo{c}")
            nc.sync.dma_start(wot[:], wo_d[c * 128:(c + 1) * 128, :])
            wos.append(wot)
        for s_t in range(S // 128):
            ob = ob_pool.tile([128, D], F32, name=f"ob{s_t}", tag="ob")
            for mg in range(D // 512):
                pso_t = pso.tile([128, 512], F32, name=f"po{s_t}_{mg}", tag="pso")
                for c in range(HP):
                    nc.tensor.matmul(pso_t[:],
                                     ctxTs[c][:, s_t * 128:(s_t + 1) * 128],
                                     wos[c][:, mg * 512:(mg + 1) * 512],
                                     start=(c == 0), stop=(c == HP - 1))
                nc.vector.tensor_copy(ob[:, mg * 512:(mg + 1) * 512], pso_t[:])
            nc.sync.dma_start(out_d[s_t * 128:(s_t + 1) * 128, :], ob[:])
        for p in (pso, ob_pool, wo_pool, ctxT_pool):
            p.release()

    nc.compile()
    return nc


_CACHE = {}
LAST_EXEC_NS = None


def kernel(x, k_cache, v_cache, Wq, bq, Wk, bk, Wv, bv, Wo, bo, pos):
    global LAST_EXEC_NS
    pos = int(pos)

    def f32(a):
        return np.ascontiguousarray(np.asarray(a), dtype=np.float32)

    x = f32(x)
    k_cache, v_cache = f32(k_cache), f32(v_cache)
    Wq, Wk, Wv, Wo = f32(Wq), f32(Wk), f32(Wv), f32(Wo)
    bq, bk, bv, bo = f32(bq), f32(bk), f32(bv), f32(bo)

    xT = np.ascontiguousarray(x[0].T)              # [D, S]
    xl = np.ascontiguousarray(
        x[0, -1].reshape(DC, 128).T.astype(ml_dtypes.bfloat16))
    in_maps = []
    for i in range(NCORES):
        hs = slice(i * HP, (i + 1) * HP)
        in_maps.append({
            "xT": xT,
            "wq": np.ascontiguousarray(Wq[hs]),
            "wkv": np.ascontiguousarray(np.concatenate([
                Wk[hs].transpose(1, 0, 2).reshape(D, HP * DK),
                Wv[hs].transpose(1, 0, 2).reshape(D, HP * DK)],
                axis=1).astype(ml_dtypes.bfloat16)),
            "xl": xl,
            "bq": np.ascontiguousarray(bq[hs].reshape(HP, DK, 1)),
            "bkv": np.ascontiguousarray(np.concatenate(
                [bk[hs].reshape(-1), bv[hs].reshape(-1)])[None, :]),
            "kT": np.ascontiguousarray(k_cache[hs, :pos, :].transpose(0, 2, 1)),
            "v": np.ascontiguousarray(v_cache[hs, :pos, :]),
            "wo": np.ascontiguousarray(Wo[i * HP * DK:(i + 1) * HP * DK]),
        })

    if pos not in _CACHE:
        _CACHE[pos] = build(pos)
    nc = _CACHE[pos]

    res = run_bass_kernel_spmd(nc, in_maps, core_ids=list(range(NCORES)))
    LAST_EXEC_NS = res.exec_time_ns

    acc = np.zeros((S, D), np.float64)
    for r in res.results:
        acc += r["out"]
    out = (acc + bo.astype(np.float64)).astype(np.float32)
    return out[None]


# revision 14
# speedup vs baseline: 1.2530x; 1.0104x over previous
"""Trainium2 Bass kernel for CachedMultiHeadedAttention (tensor-parallel over heads).

Sharding: 8 cores x 4 heads. Each core computes Q projection + attention for
its 4 heads, then a partial output projection against its 512 rows of Wo.
Host sums the 8 partial outputs (the "all-reduce" done at unshard time) and
adds bo.

Device-side layouts are chosen so NO on-chip transposes are needed:
  - x is passed pre-transposed (xT [D, S]) so contraction dims land on
    SBUF partitions for every matmul.
  - k_cache is passed pre-transposed per head (kT [DK, pos]).
  - The reference's softmax quirk (softmax over the QUERY axis) maps to
    scores^T tiles [l_part, s_free]: one fused ACT pass does exp + row-sum.
    The 1/sum normalization is folded into V rows (8x less data than the
    weight matrix).
Matmuls run as float32r (full PE rate; plain fp32 matmul is 1/4 rate).
The k_new/v_new projections (rank-1 work) run in bf16 — they only affect one
of the 4096 cache rows, so their rounding is negligible in the output.

DMAs are consolidated into few large transfers: each dma_start costs ~625ns
on the single HWDGE queue, which was the dominant bottleneck of the naive
version (557 DMAs = 348us of serialized HWDGE time).
"""

import math

import numpy as np
import ml_dtypes

import concourse.bass as bass
import concourse.mybir as mybir
import concourse.tile as tile
from concourse import bacc
from concourse.bass_utils import run_bass_kernel_spmd

F32 = mybir.dt.float32
F32R = mybir.dt.float32r
BF16 = mybir.dt.bfloat16
AF = mybir.ActivationFunctionType

H, D, DK, S = 32, 4096, 128, 1024
NCORES = 8
HP = H // NCORES          # heads per core
DC = D // 128             # contraction chunks for d_model


def build(pos: int):
    L = pos + 1
    LC = (L + 127) // 128          # number of 128-wide l tiles
    LG = (LC + 7) // 8             # l-tile groups of 8 (1024 l per group)
    INV = 1.0 / math.sqrt(DK)

    nc = bacc.Bacc("TRN2", target_bir_lowering=False, debug=False,
                   num_devices=NCORES)

    xT_d = nc.dram_tensor("xT", [D, S], F32R, kind="ExternalInput").ap()
    wq_d = nc.dram_tensor("wq", [HP, D, DK], F32R, kind="ExternalInput").ap()
    wkv_d = nc.dram_tensor("wkv", [D, 2 * HP * DK], BF16, kind="ExternalInput").ap()
    xl_d = nc.dram_tensor("xl", [128, DC], BF16, kind="ExternalInput").ap()
    bq_d = nc.dram_tensor("bq", [HP, DK, 1], F32, kind="ExternalInput").ap()
    bkv_d = nc.dram_tensor("bkv", [1, 2 * HP * DK], F32, kind="ExternalInput").ap()
    kT_d = nc.dram_tensor("kT", [HP, DK, pos], F32R, kind="ExternalInput").ap()
    v_d = nc.dram_tensor("v", [HP, pos, DK], F32R, kind="ExternalInput").ap()
    wo_d = nc.dram_tensor("wo", [HP * DK, D], F32R, kind="ExternalInput").ap()
    out_d = nc.dram_tensor("out", [S, D], F32, kind="ExternalOutput").ap()

    with tile.TileContext(nc) as tc:
        # Pools are released LIFO; ctxT survives into the output projection,
        # so it sits at the bottom of the SBUF pool stack.
        ctxT_pool = tc.alloc_tile_pool(name="ctxT", bufs=1)
        xT_pool = tc.alloc_tile_pool(name="xT", bufs=1)
        qT_pool = tc.alloc_tile_pool(name="qT", bufs=2)
        small = tc.alloc_tile_pool(name="smallp", bufs=1)
        wq_pool = tc.alloc_tile_pool(name="wqp", bufs=3)
        wkv_pool = tc.alloc_tile_pool(name="wkvp", bufs=2)
        kt_pool = tc.alloc_tile_pool(name="ktp", bufs=2)
        v_pool = tc.alloc_tile_pool(name="vp", bufs=2)
        wt_pool = tc.alloc_tile_pool(name="wtp", bufs=3)
        vs_pool = tc.alloc_tile_pool(name="vsp", bufs=4)
        ss_pool = tc.alloc_tile_pool(name="ssp", bufs=8)

        psq = tc.alloc_tile_pool(name="psq", bufs=1, space="PSUM")
        kvn_pool = tc.alloc_tile_pool(name="kvn", bufs=1, space="PSUM")
        pss = psc = None   # allocated after kvn_pool is released

        # resident xT tiles: 8 big tiles of 4 chunks each (4KB descriptors)
        xbig = []
        for gx in range(DC // 4):
            xt = xT_pool.tile([128, 4 * S], F32R, name=f"xt{gx}", tag=f"xt{gx}")
            nc.sync.dma_start(
                xt[:], xT_d[gx * 512:(gx + 1) * 512, :].rearrange(
                    "(i p) s -> p i s", p=128))
            xbig.append(xt)

        def xsl(c, lo, sz):
            return xbig[c // 4][:, (c % 4) * S + lo:(c % 4) * S + lo + sz]

        ctxTs = [ctxT_pool.tile([128, S], F32R, name=f"cT{h}", tag=f"cT{h}")
                 for h in range(HP)]

        # biased k_new|v_new rows for all heads ([1, 2*HP*DK])
        kvrow = small.tile([1, 2 * HP * DK], F32R, name="kvrow", tag="kvrow")
        bkv_t = small.tile([1, 2 * HP * DK], F32, name="bkvt", tag="bkvt")
        nc.sync.dma_start(bkv_t[:], bkv_d[:])
        # x_last chunks in bf16: column c = x[-1, c*128:(c+1)*128]
        xl_t = small.tile([128, DC], BF16, name="xlt", tag="xlt")
        nc.sync.dma_start(xl_t[:], xl_d[:])

        # k_new/v_new psum accumulators (rows, all heads)
        kn_t = kvn_pool.tile([1, HP * DK], F32, name="knr", tag="kn", bufs=1)
        vn_t = kvn_pool.tile([1, HP * DK], F32, name="vnr", tag="vn", bufs=1)

        npos_g = pos // 1024            # l-group containing the new entry
        npos_j = (pos % 1024) // 128    # l-tile within that group
        for h in range(HP):
            # ---------- Q projection for head h ----------
            psq_t = psq.tile([128, S], F32, name=f"psq{h}", tag="psq")
            wqts = []
            for gw in range(DC // 4):   # 8 weight groups of 4 chunks
                wqt = wq_pool.tile([128, 4 * DK], F32R,
                                   name=f"wq{h}_{gw}", tag="wq")
                nc.sync.dma_start(
                    wqt[:], wq_d[h, gw * 512:(gw + 1) * 512, :].rearrange(
                        "(i p) k -> p i k", p=128))
                wqts.append(wqt)
            for c in range(DC):
                lhs = wqts[c // 4][:, (c % 4) * DK:(c % 4 + 1) * DK]
                st, sp = (c == 0), (c == DC - 1)
                nc.tensor.matmul(psq_t[:, 0:512], lhs, xsl(c, 0, 512),
                                 start=st, stop=sp)
                nc.tensor.matmul(psq_t[:, 512:1024], lhs, xsl(c, 512, 512),
                                 start=st, stop=sp)
            # qT = psq + bq (per-partition bias) on DVE
            bq_t = ss_pool.tile([128, 1], F32, name=f"bq{h}", tag="bq", bufs=2)
            nc.sync.dma_start(bq_t[:], bq_d[h])
            qT_t = qT_pool.tile([128, S], F32R, name=f"qT{h}", tag="qT")
            nc.vector.tensor_scalar_add(qT_t[:], psq_t[:], bq_t[:])

            if h == 0:
                # rank-1 k_new/v_new projections (bf16) — emitted before the
                # S loop so the kvrow write precedes all kvrow readers in
                # trace order (Tile tracks deps in emission order).
                for c in range(DC):
                    wkvt = wkv_pool.tile([128, 2 * HP * DK], BF16,
                                         name=f"wkv{c}", tag="wkv")
                    nc.sync.dma_start(wkvt[:], wkv_d[c * 128:(c + 1) * 128, :])
                    st, sp = (c == 0), (c == DC - 1)
                    nc.tensor.matmul(kn_t[0:1, :], xl_t[:, c:c + 1],
                                     wkvt[:, 0:HP * DK], start=st, stop=sp)
                    nc.tensor.matmul(vn_t[0:1, :], xl_t[:, c:c + 1],
                                     wkvt[:, HP * DK:], start=st, stop=sp)
                nc.vector.tensor_add(kvrow[0:1, 0:HP * DK], kn_t[:],
                                     bkv_t[0:1, 0:HP * DK])
                nc.vector.tensor_add(kvrow[0:1, HP * DK:], vn_t[:],
                                     bkv_t[0:1, HP * DK:])
                kvn_pool.release()
                pss = tc.alloc_tile_pool(name="pss", bufs=2, space="PSUM")
                psc = tc.alloc_tile_pool(name="psc", bufs=1, space="PSUM")

            # ---------- attention for head h over l tiles ----------
            psc_t = psc.tile([128, S], F32, name=f"psc{h}", tag="psc")
            kt8 = v8 = None
            for lt in range(LC):
                g, j = lt // 8, lt % 8
                if j == 0:
                    # load l-group g: keys (16KB bursts) and values (512B)
                    g0 = g * 1024
                    gl = min(1024, L - g0)            # valid l in group
                    gc = max(0, min(1024, pos - g0))  # from cache
                    kt8 = kt_pool.tile([128, 1024], F32R,
                                       name=f"kt{h}_{g}", tag="kt")
                    if gc > 0:
                        nc.sync.dma_start(kt8[:, 0:gc], kT_d[h, :, g0:g0 + gc])
                    if gl > gc:   # new entry column
                        nc.sync.dma_start(kt8[:, gc:gc + 1],
                                          kvrow[0:1, h * DK:(h + 1) * DK])
                    if gl < 1024:
                        nc.vector.memset(kt8[:, gl:1024], 0.0)
                    v8 = v_pool.tile([128, 1024], F32R,
                                     name=f"v{h}_{g}", tag="v")
                    fc = gc // 128                    # full cache chunks
                    if fc > 0:
                        nc.sync.dma_start(
                            v8[:, 0:fc * 128],
                            v_d[h, g0:g0 + fc * 128, :].rearrange(
                                "(i p) k -> p i k", p=128))
                    rem = gc - fc * 128               # partial cache chunk
                    if rem > 0:
                        nc.sync.dma_start(
                            v8[0:rem, fc * 128:(fc + 1) * 128],
                            v_d[h, g0 + fc * 128:g0 + gc, :])
                    if gl > gc:   # new entry row
                        nc.sync.dma_start(
                            v8[gc % 128:gc % 128 + 1,
                               (gc // 128) * 128:(gc // 128 + 1) * 128],
                            kvrow[0:1, HP * DK + h * DK:HP * DK + (h + 1) * DK])
                    if gl < 1024:
                        for cc in range(gl // 128, 8):
                            lo = max(gl - cc * 128, 0)
                            if lo < 128:
                                nc.vector.memset(v8[lo:128, cc * 128:(cc + 1) * 128], 0.0)

                ps = pss.tile([128, 1024], F32, name=f"ps_{h}_{lt}", tag="pss")
                ksl = kt8[:, j * 128:(j + 1) * 128]
                nc.tensor.matmul(ps[:, 0:512], ksl, qT_t[:, 0:512])
                nc.tensor.matmul(ps[:, 512:1024], ksl, qT_t[:, 512:1024])

                wt = wt_pool.tile([128, 1024], F32R, name=f"wt_{h}_{lt}", tag="wt")
                ssum = ss_pool.tile([128, 1], F32, name=f"ss_{h}_{lt}", tag="ssum")
                nc.scalar.activation(wt[:], ps[:], AF.Exp, scale=INV, accum_out=ssum[:])
                rec = ss_pool.tile([128, 1], F32, name=f"rc_{h}_{lt}", tag="rec")
                nc.vector.reciprocal(rec[:], ssum[:])
                vst = vs_pool.tile([128, DK], F32R, name=f"vs{h}_{lt}", tag="vs")
                nc.vector.tensor_scalar_mul(vst[:], v8[:, j * 128:(j + 1) * 128], rec[:])

                st, sp = (lt == 0), (lt == LC - 1)
                nc.tensor.matmul(psc_t[:, 0:512], vst[:], wt[:, 0:512],
                                 start=st, stop=sp)
                nc.tensor.matmul(psc_t[:, 512:1024], vst[:], wt[:, 512:1024],
                                 start=st, stop=sp)
            nc.vector.tensor_copy(ctxTs[h][:], psc_t[:])

        # release attention-phase pools before the output projection (LIFO)
        for p in (psc, pss, psq,
                  ss_pool, vs_pool, wt_pool, v_pool, kt_pool,
                  wkv_pool, wq_pool, small, qT_pool, xT_pool):
            p.release()

        # ---------- output projection: out[s, m] partial ----------
        # Wo fully resident in the space freed by xT; one 16KB-burst output
        # DMA per s-tile.
        wo_pool = tc.alloc_tile_pool(name="wop", bufs=1)
        ob_pool = tc.alloc_tile_pool(name="obp", bufs=2)
        pso = tc.alloc_tile_pool(name="pso", bufs=4, space="PSUM")
        wos = []
        for c in range(HP):
            wot = wo_pool.tile([128, D], F32R, name=f"wo{c}", tag=f"wo{c}")
            nc.sync.dma_start(wot[:], wo_d[c * 128:(c + 1) * 128, :])
            wos.append(wot)
        for s_t in range(S // 128):
            ob = ob_pool.tile([128, D], F32, name=f"ob{s_t}", tag="ob")
            for mg in range(D // 512):
                pso_t = pso.tile([128, 512], F32, name=f"po{s_t}_{mg}", tag="pso")
                for c in range(HP):
                    nc.tensor.matmul(pso_t[:],
                                     ctxTs[c][:, s_t * 128:(s_t + 1) * 128],
                                     wos[c][:, mg * 512:(mg + 1) * 512],
                                     start=(c == 0), stop=(c == HP - 1))
                nc.vector.tensor_copy(ob[:, mg * 512:(mg + 1) * 512], pso_t[:])
            nc.sync.dma_start(out_d[s_t * 128:(s_t + 1) * 128, :], ob[:])
        for p in (pso, ob_pool, wo_pool, ctxT_pool):
            p.release()

    nc.compile()
    return nc


_CACHE = {}
LAST_EXEC_NS = None


def kernel(x, k_cache, v_cache, Wq, bq, Wk, bk, Wv, bv, Wo, bo, pos):
    global LAST_EXEC_NS
    pos = int(pos)

    def f32(a):
        return np.ascontiguousarray(np.asarray(a), dtype=np.float32)

    x = f32(x)
    k_cache, v_cache = f32(k_cache), f32(v_cache)
    Wq, Wk, Wv, Wo = f32(Wq), f32(Wk), f32(Wv), f32(Wo)
    bq, bk, bv, bo = f32(bq), f32(bk), f32(bv), f32(bo)

    xT = np.ascontiguousarray(x[0].T)              # [D, S]
    xl = np.ascontiguousarray(
        x[0, -1].reshape(DC, 128).T.astype(ml_dtypes.bfloat16))
    in_maps = []
    for i in range(NCORES):
        hs = slice(i * HP, (i + 1) * HP)
        in_maps.append({
            "xT": xT,
            "wq": np.ascontiguousarray(Wq[hs]),
            "wkv": np.ascontiguousarray(np.concatenate([
                Wk[hs].transpose(1, 0, 2).reshape(D, HP * DK),
                Wv[hs].transpose(1, 0, 2).reshape(D, HP * DK)],
                axis=1).astype(ml_dtypes.bfloat16)),
            "xl": xl,
            "bq": np.ascontiguousarray(bq[hs].reshape(HP, DK, 1)),
            "bkv": np.ascontiguousarray(np.concatenate(
                [bk[hs].reshape(-1), bv[hs].reshape(-1)])[None, :]),
            "kT": np.ascontiguousarray(k_cache[hs, :pos, :].transpose(0, 2, 1)),
            "v": np.ascontiguousarray(v_cache[hs, :pos, :]),
            "wo": np.ascontiguousarray(Wo[i * HP * DK:(i + 1) * HP * DK]),
        })

    if pos not in _CACHE:
        _CACHE[pos] = build(pos)
    nc = _CACHE[pos]

    res = run_bass_kernel_spmd(nc, in_maps, core_ids=list(range(NCORES)))
    LAST_EXEC_NS = res.exec_time_ns

    acc = np.zeros((S, D), np.float64)
    for r in res.results:
        acc += r["out"]
    out = (acc + bo.astype(np.float64)).astype(np.float32)
    return out[None]
